# revision 2
# baseline (speedup 1.0000x reference)
"""Anisotropic 3D SSIM loss on 8 TRN2 NeuronCores.

Input: img1, img2 of shape (2, 1, 128, 256, 256) f32.
Reference: separable Gaussian blur (27/5/5 taps on D/H/W, sigma=1.5),
SSIM map, loss = 1 - mean(ssim_map).

 - Shard H=256 across 8 cores: 32 output rows + 2-row zero halo (host pads).
 - D=128 on the SBUF partition axis; 4 blur volumes (linearity).
 - D+H conv fused: 5 PSUM-accumulated N=512 bf16 matmuls with stationary
   gy[j]*BandD; W conv: 5 shifted scaled-identity matmuls.
 - Processed in 16-row halves, double-buffered, raw inputs streamed through
   small chunk tiles; SSIM formula on DVE/ACT; partial sums out, host combines.
"""

import numpy as np

N_CORES = 8
NB, D, H, W = 2, 128, 256, 256
HS = H // N_CORES          # 32 output rows per core
HH = HS + 4                # 36 input rows with halo
SIGMA = 1.5
WZ, WY, WX = 27, 5, 5
C1 = np.float32(0.01 ** 2)
C2 = np.float32(0.03 ** 2)
NCOL = 32


def _gauss1d(n):
    x = np.arange(n, dtype=np.float64)
    g = np.exp(-(x - n // 2) ** 2 / (2.0 * SIGMA ** 2))
    return (g / g.sum()).astype(np.float32)


def _build_consts():
    import ml_dtypes
    gz, gy, gx = _gauss1d(WZ), _gauss1d(WY), _gauss1d(WX)
    k = np.arange(D)[:, None]
    m = np.arange(D)[None, :]
    idx = k - m + WZ // 2
    band = np.where((idx >= 0) & (idx < WZ), gz[np.clip(idx, 0, WZ - 1)], 0.0)
    band = band.astype(np.float32)
    wd = np.concatenate([band * gy[j] for j in range(WY)], axis=1)
    eye = np.eye(D, dtype=np.float32)
    wi = np.concatenate([eye * gx[j] for j in range(WX)], axis=1)
    return (np.ascontiguousarray(wd.astype(ml_dtypes.bfloat16)),
            np.ascontiguousarray(wi.astype(ml_dtypes.bfloat16)))


def _build_bass():
    import concourse.bass as bass
    import concourse.bacc as bacc
    import concourse.mybir as mybir
    from concourse import tile

    f32 = mybir.dt.float32
    bf16 = mybir.dt.bfloat16
    Alu = mybir.AluOpType
    Act = mybir.ActivationFunctionType

    nc = bacc.Bacc(None, target_bir_lowering=False)

    FL = HH * W
    img1 = nc.declare_dram_parameter("img1", [NB, D, FL], f32, isOutput=False)
    img2 = nc.declare_dram_parameter("img2", [NB, D, FL], f32, isOutput=False)
    wd = nc.declare_dram_parameter("wd", [D, WY * D], bf16, isOutput=False)
    wi = nc.declare_dram_parameter("wi", [D, WX * D], bf16, isOutput=False)
    out = nc.declare_dram_parameter("out", [D, NCOL], f32, isOutput=True)

    HB = 2                  # out rows per psum block
    HHALF = 16              # out rows per half
    HIN = HHALF + 4         # 20 input rows per half
    NHB = HHALF // HB       # 8 blocks per half
    FB = HB * W             # 512
    RC = 5                  # raw chunk rows
    NRC = HIN // RC
    FRC = RC * W

    with tile.TileContext(nc) as tc:
        with (
            tc.tile_pool(name="consts", bufs=1) as cpool,
            tc.tile_pool(name="raw", bufs=4) as rawpool,
            tc.tile_pool(name="prod", bufs=2) as prodpool,
            tc.tile_pool(name="dh", bufs=4) as dhpool,
            tc.tile_pool(name="ftmp", bufs=2) as fpool,
            tc.tile_pool(name="pdh", bufs=2, space=bass.MemorySpace.PSUM) as pdh,
            tc.tile_pool(name="pw", bufs=6, space=bass.MemorySpace.PSUM) as pw,
        ):
            wd_b = cpool.tile([D, WY * D], bf16, tag="wdb")
            wi_b = cpool.tile([D, WX * D], bf16, tag="wib")
            nc.sync.dma_start(wd_b[:], wd[:, :])
            nc.sync.dma_start(wi_b[:], wi[:, :])

            staging = cpool.tile([D, NCOL], f32, tag="stage")
            bias_c1 = cpool.tile([D, 1], f32, tag="bc1")
            bias_c2 = cpool.tile([D, 1], f32, tag="bc2")
            nc.vector.memset(bias_c1[:], float(C1))
            nc.vector.memset(bias_c2[:], float(C2))

            for n in range(NB):
                for half in range(2):
                    in0 = half * HHALF      # first input row of the half
                    FH = HIN * W
                    v1 = prodpool.tile([D, FH], bf16, tag="v1")
                    v2 = prodpool.tile([D, FH], bf16, tag="v2")
                    vs = prodpool.tile([D, FH], bf16, tag="vs")
                    vp = prodpool.tile([D, FH], bf16, tag="vp")
                    for ci in range(NRC):
                        r1 = rawpool.tile([D, FRC], f32, tag="r1")
                        r2 = rawpool.tile([D, FRC], f32, tag="r2")
                        off = (in0 + ci * RC) * W
                        nc.sync.dma_start(r1[:], img1[n, :, off:off + FRC])
                        nc.sync.dma_start(r2[:], img2[n, :, off:off + FRC])
                        s = slice(ci * FRC, (ci + 1) * FRC)
                        # vs = (r1+r2)^2 - 2*r1*r2  (= r1^2 + r2^2)
                        nc.vector.tensor_tensor(vp[:, s], r1[:], r2[:], Alu.mult)
                        nc.vector.tensor_tensor(vs[:, s], r1[:], r2[:], Alu.add)
                        nc.scalar.activation(vs[:, s], vs[:, s], Act.Square)
                        nc.vector.scalar_tensor_tensor(
                            vs[:, s], vp[:, s], -2.0, vs[:, s],
                            Alu.mult, Alu.add)
                        nc.scalar.activation(v1[:, s], r1[:], Act.Copy)
                        nc.vector.tensor_copy(v2[:, s], r2[:])
                    vols = [v1, v2, vs, vp]

                    for hb in range(NHB):
                        dhts = []
                        for v in range(4):
                            ps = pdh.tile([D, FB], f32, tag="pdh")
                            for j in range(WY):
                                off = (hb * HB + j) * W
                                nc.tensor.matmul(
                                    ps[:],
                                    wd_b[:, j * D:(j + 1) * D],
                                    vols[v][:, off:off + FB],
                                    start=(j == 0),
                                    stop=(j == WY - 1),
                                )
                            dht = dhpool.tile([D, FB], bf16, tag=f"dh{v}")
                            nc.scalar.activation(dht[:], ps[:], Act.Copy)
                            dhts.append(dht)

                        wps = []
                        for v in range(4):
                            pwv = pw.tile([D, FB], f32, tag="pw")
                            order = [2, 0, 1, 3, 4]
                            for r in range(HB):
                                for t, dw in enumerate(order):
                                    o = dw - 2
                                    lo_in, hi_in = max(0, o), W + min(0, o)
                                    lo_out = max(0, -o)
                                    width = hi_in - lo_in
                                    nc.tensor.matmul(
                                        pwv[:, r * W + lo_out:
                                            r * W + lo_out + width],
                                        wi_b[:, dw * D:(dw + 1) * D],
                                        dhts[v][:, r * W + lo_in:r * W + hi_in],
                                        start=(t == 0),
                                        stop=(t == WX - 1),
                                    )
                            wps.append(pwv)

                        b1, b2, bs, bp = (w[:] for w in wps)

                        c2t = fpool.tile([D, FB], bf16, tag="c2t")
                        t1 = fpool.tile([D, FB], bf16, tag="t1")
                        m12 = fpool.tile([D, FB], bf16, tag="m12")
                        den1 = fpool.tile([D, FB], bf16, tag="den1")
                        den2 = fpool.tile([D, FB], bf16, tag="den2")
                        dd = fpool.tile([D, FB], bf16, tag="dd")
                        rec = fpool.tile([D, FB], f32, tag="rec")
                        scr = fpool.tile([D, FB], f32, tag="scr")

                        nc.scalar.activation(c2t[:], b2, Act.Copy)
                        nc.scalar.activation(t1[:], b1, Act.Square)
                        nc.vector.tensor_tensor(m12[:], b1, c2t[:], Alu.mult)
                        # c2t <- t2 = c2t^2 (in place)
                        nc.scalar.activation(c2t[:], c2t[:], Act.Square)
                        # den1 = (t1 + C1) + t2
                        nc.vector.scalar_tensor_tensor(
                            den1[:], t1[:], float(C1), c2t[:], Alu.add, Alu.add)
                        # den2 = (bs + (C1+C2)) - den1
                        nc.vector.scalar_tensor_tensor(
                            den2[:], bs, float(C1 + C2), den1[:],
                            Alu.add, Alu.subtract)
                        # dd = bp - m12 ; dd <- num2 = 2*dd + C2
                        nc.vector.tensor_tensor(dd[:], bp, m12[:], Alu.subtract)
                        nc.scalar.activation(dd[:], dd[:], Act.Identity,
                                             bias=bias_c2[:], scale=2.0)
                        # m12 <- num1 = 2*m12 + C1
                        nc.scalar.activation(m12[:], m12[:], Act.Identity,
                                             bias=bias_c1[:], scale=2.0)
                        # den1 <- den = den1 * den2
                        nc.vector.tensor_tensor(den1[:], den1[:], den2[:],
                                                Alu.mult)
                        nc.vector.reciprocal(rec[:], den1[:])
                        # m12 <- num = num1 * num2
                        nc.vector.tensor_tensor(m12[:], m12[:], dd[:], Alu.mult)
                        col = n * 16 + half * NHB + hb
                        nc.vector.scalar_tensor_tensor(
                            scr[:], m12[:], 1.0, rec[:], Alu.mult, Alu.mult,
                            accum_out=staging[:, col:col + 1],
                        )

            nc.sync.dma_start(out[:, :], staging[:])

    nc.compile()
    return nc


_CACHED = {}


def _make_in_maps(img1: np.ndarray, img2: np.ndarray) -> list:
    i1 = np.asarray(img1, dtype=np.float32).reshape(NB, D, H, W)
    i2 = np.asarray(img2, dtype=np.float32).reshape(NB, D, H, W)

    wd, wi = _build_consts()

    p1 = np.pad(i1, ((0, 0), (0, 0), (2, 2), (0, 0)))
    p2 = np.pad(i2, ((0, 0), (0, 0), (2, 2), (0, 0)))
    in_maps = []
    for c in range(N_CORES):
        lo = c * HS
        in_maps.append({
            "img1": np.ascontiguousarray(
                p1[:, :, lo:lo + HH, :]).reshape(NB, D, HH * W),
            "img2": np.ascontiguousarray(
                p2[:, :, lo:lo + HH, :]).reshape(NB, D, HH * W),
            "wd": wd,
            "wi": wi,
        })
    return in_maps


def _combine(results: list) -> np.float32:
    total = np.float64(0.0)
    for r in results:
        total += np.sum(r["out"].astype(np.float64))
    mean = total / (NB * D * H * W)
    return np.float32(1.0 - mean)


def kernel(img1: np.ndarray, img2: np.ndarray) -> np.ndarray:
    from concourse.bass_utils import run_bass_kernel_spmd

    in_maps = _make_in_maps(img1, img2)

    if "nc" not in _CACHED:
        _CACHED["nc"] = _build_bass()
    nc = _CACHED["nc"]

    res = run_bass_kernel_spmd(nc, in_maps, core_ids=list(range(N_CORES)))
    return _combine(res.results)



# revision 5
# speedup vs baseline: 1.2819x; 1.2819x over previous
"""Anisotropic 3D SSIM loss on 8 TRN2 NeuronCores.

Input: img1, img2 of shape (2, 1, 128, 256, 256) f32.
Reference: separable Gaussian blur (27/5/5 taps on D/H/W, sigma=1.5),
SSIM map, loss = 1 - mean(ssim_map).

v2 design:
 - Shard H=256 across 8 cores: 32 output rows + 2-row zero halo (host pads).
 - a/b basis: a=img1+img2, b=img1-img2 (then mu1*mu2, mu1^2+mu2^2, E[xy],
   E[x^2]+E[y^2] are linear combos of blur(a), blur(b), blur(a^2), blur(b^2)).
 - Volumes quantized to fp8e4m3; D+H conv = 3 PSUM-accumulated matmuls per
   2-row block (2x DoubleRow pairing H-taps (0,2),(1,3) + 1 normal for tap 4)
   with stationary fp8 gy[j]*BandD weights.
 - W conv: 10 shifted scaled-identity bf16 matmuls on the bf16 D+H output
   (0.5 factor for the second-moment volumes folded into the W weights).
 - SSIM formula on DVE in bf16 (2x mode); reciprocal via ACT Ln+Exp
   (single act-table set: natural_log_exp has ln/exp/square/copy/identity).
 - Per-block per-partition partial sums accumulated on-chip; host combines.
"""

import numpy as np

N_CORES = 8
NB, D, H, W = 2, 128, 256, 256
HS = H // N_CORES          # 32 output rows per core
HH = HS + 4                # 36 input rows with halo
SIGMA = 1.5
WZ, WY, WX = 27, 5, 5
C1 = float(0.01 ** 2)
C2 = float(0.03 ** 2)
C12 = C1 + C2
NCOL = 32
RSQRT2 = float(np.float32(0.7071067811865476))


def _gauss1d(n):
    x = np.arange(n, dtype=np.float64)
    g = np.exp(-(x - n // 2) ** 2 / (2.0 * SIGMA ** 2))
    return (g / g.sum()).astype(np.float32)


def _build_consts():
    import ml_dtypes
    gz, gy, gx = _gauss1d(WZ), _gauss1d(WY), _gauss1d(WX)
    k = np.arange(D)[:, None]
    m = np.arange(D)[None, :]
    idx = k - m + WZ // 2
    band = np.where((idx >= 0) & (idx < WZ), gz[np.clip(idx, 0, WZ - 1)], 0.0)
    band = band.astype(np.float32)
    # D+H weights, fp8, DoubleRow pair layout: [gy0*B | gy2*B | gy1*B | gy3*B | gy4*B]
    wd = np.concatenate([band * gy[j] for j in (0, 2, 1, 3, 4)], axis=1)
    eye = np.eye(D, dtype=np.float32)
    wi = np.concatenate([eye * gx[j] for j in range(WX)], axis=1)
    return (np.ascontiguousarray(wd.astype(ml_dtypes.float8_e4m3fn)),
            np.ascontiguousarray(wi.astype(ml_dtypes.bfloat16)),
            np.ascontiguousarray((0.5 * wi).astype(ml_dtypes.bfloat16)))


def _build_bass():
    import concourse.bass as bass
    import concourse.bacc as bacc
    import concourse.mybir as mybir
    from concourse import tile

    f32 = mybir.dt.float32
    bf16 = mybir.dt.bfloat16
    fp8 = mybir.dt.float8e4
    Alu = mybir.AluOpType
    Act = mybir.ActivationFunctionType
    DR = mybir.MatmulPerfMode.DoubleRow

    nc = bacc.Bacc(None, target_bir_lowering=False)

    FL = HH * W
    img1 = nc.declare_dram_parameter("img1", [NB, D, FL], f32, isOutput=False)
    img2 = nc.declare_dram_parameter("img2", [NB, D, FL], f32, isOutput=False)
    wd = nc.declare_dram_parameter("wd", [D, 5 * D], fp8, isOutput=False)
    wi = nc.declare_dram_parameter("wi", [D, WX * D], bf16, isOutput=False)
    wis = nc.declare_dram_parameter("wis", [D, WX * D], bf16, isOutput=False)
    out = nc.declare_dram_parameter("out", [D, NCOL], f32, isOutput=True)

    HB = 2                  # out rows per psum block
    HHALF = 16              # out rows per half-unit
    HIN = HHALF + 4         # 20 input rows per half-unit
    NHB = HHALF // HB       # 8 blocks per half-unit
    FB = HB * W             # 512
    RC = 5                  # raw chunk rows
    NRC = HIN // RC
    FRC = RC * W

    with tile.TileContext(nc) as tc:
        with (
            tc.tile_pool(name="consts", bufs=1) as cpool,
            tc.tile_pool(name="raw", bufs=4) as rawpool,
            tc.tile_pool(name="vols", bufs=2) as vpool,
            tc.tile_pool(name="dh", bufs=8) as dhpool,
            tc.tile_pool(name="ftmp", bufs=2) as fpool,
            tc.tile_pool(name="pdh", bufs=4, space=bass.MemorySpace.PSUM) as pdh,
            tc.tile_pool(name="pw", bufs=4, space=bass.MemorySpace.PSUM) as pw,
        ):
            wd_t = cpool.tile([D, 5 * D], fp8, tag="wd")
            wi_t = cpool.tile([D, WX * D], bf16, tag="wi")
            wis_t = cpool.tile([D, WX * D], bf16, tag="wis")
            nc.sync.dma_start(wd_t[:], wd[:, :])
            nc.sync.dma_start(wi_t[:], wi[:, :])
            nc.sync.dma_start(wis_t[:], wis[:, :])

            staging = cpool.tile([D, NCOL], f32, tag="stage")

            for n in range(NB):
                for half in range(2):
                    in0 = half * HHALF      # first input row of the half
                    FH = HIN * W
                    va = vpool.tile([D, FH], fp8, tag="va")
                    vb = vpool.tile([D, FH], fp8, tag="vb")
                    va2 = vpool.tile([D, FH], fp8, tag="va2")
                    vb2 = vpool.tile([D, FH], fp8, tag="vb2")
                    for ci in range(NRC):
                        r1 = rawpool.tile([D, FRC], f32, tag="r1")
                        r2 = rawpool.tile([D, FRC], f32, tag="r2")
                        off = (in0 + ci * RC) * W
                        nc.sync.dma_start(r1[:], img1[n, :, off:off + FRC])
                        nc.sync.dma_start(r2[:], img2[n, :, off:off + FRC])
                        s = slice(ci * FRC, (ci + 1) * FRC)
                        nc.vector.tensor_tensor(va[:, s], r1[:], r2[:], Alu.add)
                        nc.vector.tensor_tensor(vb[:, s], r1[:], r2[:],
                                                Alu.subtract)
                        nc.scalar.activation(va2[:, s], va[:, s], Act.Square)
                        nc.scalar.activation(vb2[:, s], vb[:, s], Act.Square)
                    vols = [va, vb, va2, vb2]
                    wts = [wi_t, wi_t, wis_t, wis_t]

                    for hb in range(NHB):
                        base = hb * HB
                        # D+H conv: 2 DoubleRow MMs (taps 0+2, 1+3) + tap 4
                        dhts = []
                        for v in range(4):
                            ps = pdh.tile([D, FB], f32, tag="pdh")
                            for g, j0 in enumerate((0, 1)):
                                o = (base + j0) * W
                                rhs = vols[v][:, o:o + 2 * FB].rearrange(
                                    "p (two n) -> p two n", two=2)
                                lhsT = wd_t[:, g * 2 * D:(g + 1) * 2 * D
                                            ].rearrange(
                                    "p (two m) -> p two m", two=2)
                                nc.tensor.matmul(ps[:], lhsT, rhs,
                                                 start=(g == 0), stop=False,
                                                 perf_mode=DR)
                            o = (base + 4) * W
                            nc.tensor.matmul(ps[:], wd_t[:, 4 * D:5 * D],
                                             vols[v][:, o:o + FB],
                                             start=False, stop=True)
                            dht = dhpool.tile([D, FB], bf16, tag=f"dh{v}")
                            nc.scalar.activation(dht[:], ps[:], Act.Copy)
                            dhts.append(dht)

                        # W conv: 5 shifted scaled-identity matmuls per row
                        wps = []
                        order = [2, 0, 1, 3, 4]
                        for v in range(4):
                            pwv = pw.tile([D, FB], f32, tag="pw")
                            wv = wts[v]
                            for r in range(HB):
                                for t, dw in enumerate(order):
                                    o = dw - 2
                                    lo_in, hi_in = max(0, o), W + min(0, o)
                                    lo_out = max(0, -o)
                                    width = hi_in - lo_in
                                    nc.tensor.matmul(
                                        pwv[:, r * W + lo_out:
                                            r * W + lo_out + width],
                                        wv[:, dw * D:(dw + 1) * D],
                                        dhts[v][:, r * W + lo_in:r * W + hi_in],
                                        start=(t == 0),
                                        stop=(t == WX - 1),
                                    )
                            wps.append(pwv)

                        A, B, Sa, Sb = (p[:] for p in wps)

                        P = fpool.tile([D, FB], bf16, tag="P")
                        Q = fpool.tile([D, FB], bf16, tag="Q")
                        sbs = fpool.tile([D, FB], bf16, tag="sbs")
                        num1 = fpool.tile([D, FB], bf16, tag="num1")
                        den1 = fpool.tile([D, FB], bf16, tag="den1")
                        t2 = fpool.tile([D, FB], bf16, tag="t2")
                        u = fpool.tile([D, FB], bf16, tag="u")
                        den2 = fpool.tile([D, FB], bf16, tag="den2")
                        num2 = fpool.tile([D, FB], bf16, tag="num2")
                        num = fpool.tile([D, FB], bf16, tag="num")
                        den = fpool.tile([D, FB], bf16, tag="den")
                        lden = fpool.tile([D, FB], bf16, tag="lden")
                        rec = fpool.tile([D, FB], bf16, tag="rec")
                        scr = fpool.tile([D, FB], f32, tag="scr")

                        # P = A^2/2, Q = B^2/2  (mu products basis)
                        nc.scalar.activation(P[:], A, Act.Square, scale=RSQRT2)
                        nc.scalar.activation(Q[:], B, Act.Square, scale=RSQRT2)
                        # num1 = 2 mu1 mu2 + C1 = P - Q + C1
                        nc.vector.scalar_tensor_tensor(
                            num1[:], P[:], C1, Q[:], Alu.add, Alu.subtract)
                        # den1 = mu1^2 + mu2^2 + C1 = P + Q + C1
                        nc.vector.scalar_tensor_tensor(
                            den1[:], P[:], C1, Q[:], Alu.add, Alu.add)
                        # Sa, Sb already have the 0.5 factor (wis weights).
                        # DVE can read only one PSUM operand per op - stage Sb.
                        nc.scalar.activation(sbs[:], Sb, Act.Copy)
                        nc.vector.scalar_tensor_tensor(
                            t2[:], Sa, C12, sbs[:], Alu.add, Alu.add)
                        nc.vector.scalar_tensor_tensor(
                            u[:], Sa, C12, sbs[:], Alu.add, Alu.subtract)
                        # den2 = sigma1^2 + sigma2^2 + C2
                        nc.vector.tensor_tensor(den2[:], t2[:], den1[:],
                                                Alu.subtract)
                        # num2 = 2 sigma12 + C2
                        nc.vector.tensor_tensor(num2[:], u[:], num1[:],
                                                Alu.subtract)
                        nc.vector.tensor_tensor(num[:], num1[:], num2[:],
                                                Alu.mult)
                        nc.vector.tensor_tensor(den[:], den1[:], den2[:],
                                                Alu.mult)
                        nc.scalar.activation(lden[:], den[:], Act.Ln)
                        nc.scalar.activation(rec[:], lden[:], Act.Exp,
                                             scale=-1.0)
                        col = n * 16 + half * NHB + hb
                        nc.vector.scalar_tensor_tensor(
                            scr[:], num[:], 1.0, rec[:], Alu.mult, Alu.mult,
                            accum_out=staging[:, col:col + 1],
                        )

            nc.sync.dma_start(out[:, :], staging[:])

    nc.compile()
    return nc


_CACHED = {}


def _make_in_maps(img1: np.ndarray, img2: np.ndarray) -> list:
    i1 = np.asarray(img1, dtype=np.float32).reshape(NB, D, H, W)
    i2 = np.asarray(img2, dtype=np.float32).reshape(NB, D, H, W)

    wd, wi, wis = _build_consts()

    p1 = np.pad(i1, ((0, 0), (0, 0), (2, 2), (0, 0)))
    p2 = np.pad(i2, ((0, 0), (0, 0), (2, 2), (0, 0)))
    in_maps = []
    for c in range(N_CORES):
        lo = c * HS
        in_maps.append({
            "img1": np.ascontiguousarray(
                p1[:, :, lo:lo + HH, :]).reshape(NB, D, HH * W),
            "img2": np.ascontiguousarray(
                p2[:, :, lo:lo + HH, :]).reshape(NB, D, HH * W),
            "wd": wd,
            "wi": wi,
            "wis": wis,
        })
    return in_maps


def _combine(results: list) -> np.float32:
    total = np.float64(0.0)
    for r in results:
        total += np.sum(r["out"].astype(np.float64))
    mean = total / (NB * D * H * W)
    return np.float32(1.0 - mean)


def kernel(img1: np.ndarray, img2: np.ndarray) -> np.ndarray:
    from concourse.bass_utils import run_bass_kernel_spmd

    in_maps = _make_in_maps(img1, img2)

    if "nc" not in _CACHED:
        _CACHED["nc"] = _build_bass()
    nc = _CACHED["nc"]

    res = run_bass_kernel_spmd(nc, in_maps, core_ids=list(range(N_CORES)))
    return _combine(res.results)
